# revision 4
# baseline (speedup 1.0000x reference)
"""Trainium2 Bass kernel for nn_CGBlock (gnn_message_passing).

Computation (B=256, S=512, D=128):
  c_out = c_mlp(c)                  # [B,D] MLP with BatchNorm over batch
  s_h   = s_mlp(s)                  # [B,S,D] MLP, BN stats over batch axis 0
                                    #   (independent per (seq, feature))
  s_out = s_h * c_out[:, None, :]
  agg   = max(s_out, axis=1)
  returns (s_out, agg)

Sharding: the s-MLP BatchNorm is independent per sequence position, so we
shard the SEQUENCE axis across the 8 cores (64 positions each). Every core
holds the full batch for its slice, so BN needs no cross-device collectives.
The tiny c-MLP is computed redundantly on every core.

On-device layout is feature-major ([D, seq, batch]) so the PE array can
contract over D directly; the host transposes in/out (layout prep only).

BN folding: biases b1/b2 cancel inside BN. BN+ReLU is applied as
  t = relu(h - hb),  hb = mean - be*std/g
with the remaining per-(d,s) scale kk = g/std folded into the next layer's
weights (w_next * kk per sequence position), since relu commutes with the
positive scale kk (requires g > 0, true for these inputs).
"""
import sys
import os

for _p in ('/opt/trn_rl_repo', '/root/.axon_site/_ro/trn_rl_repo'):
    if os.path.isdir(_p) and _p not in sys.path:
        sys.path.insert(0, _p)

import numpy as np
from contextlib import ExitStack

import concourse.bass as bass
import concourse.bacc as bacc
import concourse.tile as tile
import concourse.mybir as mybir
from concourse.bass_utils import run_bass_kernel_spmd

f32 = mybir.dt.float32
f32r = mybir.dt.float32r
AF = mybir.ActivationFunctionType
ALU = mybir.AluOpType

B, S, D = 256, 512, 128
NCORES = 8
S_LOC = S // NCORES          # 64 sequence positions per core
N_LOC = S_LOC * B            # 16384 columns per core
GRP_S = 8                    # sequence positions per pipeline group
NGRP = S_LOC // GRP_S        # 8 groups
GCOLS = GRP_S * B            # 2048 columns per group
EPS = 1e-5

# Tuning knobs.
NORM_DVE_MOD = ()            # s-indices (mod 4) normalized on DVE instead of ACT
GATE_DVE_MOD = (0, 2)        # 512-col chunks (mod 4) gated on DVE direct path

LAST_RESULTS = None
_CACHE = {}


def _build_nc():
    nc = bacc.Bacc("TRN2", target_bir_lowering=False, debug=False)

    xT = nc.dram_tensor("xT", [D, N_LOC], f32r, kind="ExternalInput").ap()
    cT = nc.dram_tensor("cT", [D, B], f32r, kind="ExternalInput").ap()
    # weights pre-transposed on host: [in_feature, out_feature]
    wnames = ["w_s1", "w_s2", "w_s3", "w_c1", "w_c2", "w_c3"]
    wap = {n: nc.dram_tensor(n, [D, D], f32r, kind="ExternalInput").ap()
           for n in wnames}
    # packed per-partition vectors:
    # [s_g1, s_beg1, s_g2, s_beg2, s_b3, c_g1, c_beg1, c_g2, c_beg2, c_b3, EPS]
    vecs = nc.dram_tensor("vecs", [D, 11], f32, kind="ExternalInput").ap()

    outT = nc.dram_tensor("outT", [D, N_LOC], f32, kind="ExternalOutput").ap()

    with tile.TileContext(nc) as tc, ExitStack() as ctx:
        cpool = ctx.enter_context(tc.tile_pool(name="consts", bufs=1))
        xpool = ctx.enter_context(tc.tile_pool(name="xin", bufs=3))
        t1pool = ctx.enter_context(tc.tile_pool(name="t1", bufs=2))
        t2pool = ctx.enter_context(tc.tile_pool(name="t2", bufs=2))
        sopool = ctx.enter_context(tc.tile_pool(name="sout", bufs=3))
        upool = ctx.enter_context(tc.tile_pool(name="udrain", bufs=4))
        wkpool = ctx.enter_context(tc.tile_pool(name="wk", bufs=6))
        stpool = ctx.enter_context(tc.tile_pool(name="stats", bufs=3))
        smpool = ctx.enter_context(tc.tile_pool(name="smalls", bufs=3))
        cmpool = ctx.enter_context(tc.tile_pool(name="cmlp", bufs=2))
        pspool = ctx.enter_context(tc.tile_pool(name="ps", bufs=6, space="PSUM"))
        pcpool = ctx.enter_context(tc.tile_pool(name="psc", bufs=2, space="PSUM"))

        # ---- load constants ----
        wts = {}
        for nm in wnames:
            t = cpool.tile([D, D], f32r, tag=nm)
            nc.sync.dma_start(t[:], wap[nm])
            wts[nm] = t
        ct = cpool.tile([D, B], f32r, tag="ct")
        nc.sync.dma_start(ct[:], cT)
        vt = cpool.tile([D, 11], f32, tag="vecs")
        nc.sync.dma_start(vt[:], vecs)
        s_g1 = vt[:, 0:1]; s_beg1 = vt[:, 1:2]
        s_g2 = vt[:, 2:3]; s_beg2 = vt[:, 3:4]
        s_b3 = vt[:, 4:5]
        c_g1 = vt[:, 5:6]; c_beg1 = vt[:, 6:7]
        c_g2 = vt[:, 7:8]; c_beg2 = vt[:, 8:9]
        c_b3 = vt[:, 9:10]
        eps_ap = vt[:, 10:11]

        # ---- helper: combine bn_stats (even/odd) into hb / nhb / kk ----
        def combine_stats(st, n, g_ap, beg_ap, tag):
            me = st[:, :, 1]; mo = st[:, :, 4]
            cve = st[:, :, 2]; cvo = st[:, :, 5]
            Ssum = smpool.tile([D, n], f32, tag=f"S_{tag}")
            nc.gpsimd.tensor_tensor(Ssum[:], me, mo, op=ALU.add)
            Dd = smpool.tile([D, n], f32, tag=f"D_{tag}")
            nc.gpsimd.tensor_tensor(Dd[:], me, mo, op=ALU.subtract)
            Q = smpool.tile([D, n], f32, tag=f"Q_{tag}")
            nc.gpsimd.tensor_tensor(Q[:], cve, cvo, op=ALU.add)
            D2 = smpool.tile([D, n], f32, tag=f"D2_{tag}")
            nc.gpsimd.tensor_tensor(D2[:], Dd[:], Dd[:], op=ALU.mult)
            Qs = smpool.tile([D, n], f32, tag=f"Qs_{tag}")
            nc.vector.tensor_scalar(Qs[:], Q[:], 1.0 / B, None, op0=ALU.mult)
            veps = smpool.tile([D, n], f32, tag=f"veps_{tag}")
            nc.vector.scalar_tensor_tensor(veps[:], D2[:], 0.25, Qs[:],
                                           op0=ALU.mult, op1=ALU.add)
            std = smpool.tile([D, n], f32, tag=f"std_{tag}")
            nc.scalar.activation(std[:], veps[:], AF.Sqrt, bias=eps_ap, scale=1.0)
            rstd = smpool.tile([D, n], f32, tag=f"rstd_{tag}")
            nc.vector.reciprocal(rstd[:], std[:])
            kk = smpool.tile([D, n], f32, tag=f"kk_{tag}")
            nc.vector.tensor_scalar(kk[:], rstd[:], g_ap, None, op0=ALU.mult)
            bestd = smpool.tile([D, n], f32, tag=f"bestd_{tag}")
            nc.vector.tensor_scalar(bestd[:], std[:], beg_ap, None, op0=ALU.mult)
            hb = smpool.tile([D, n], f32, tag=f"hb_{tag}")
            nc.vector.scalar_tensor_tensor(hb[:], Ssum[:], 0.5, bestd[:],
                                           op0=ALU.mult, op1=ALU.subtract)
            nhb = smpool.tile([D, n], f32, tag=f"nhb_{tag}")
            nc.vector.tensor_scalar(nhb[:], hb[:], -1.0, None, op0=ALU.mult)
            return hb, nhb, kk

        # ---- c-MLP (tiny, replicated on every core) ----
        cc = ct
        for li, (wn, g_ap, beg_ap) in enumerate(
                [("w_c1", c_g1, c_beg1), ("w_c2", c_g2, c_beg2)]):
            pc = pcpool.tile([D, B], f32)
            nc.tensor.matmul(pc[:], wts[wn][:], cc[:],
                             start=True, stop=True)
            st = stpool.tile([D, 1, 6], f32, tag=f"stc{li}")
            nc.vector.bn_stats(st[:, 0, :], pc[:])
            hb, nhb, kk = combine_stats(st, 1, g_ap, beg_ap, f"c{li}")
            nkb = smpool.tile([D, 1], f32, tag=f"nkb_c{li}")
            nc.vector.scalar_tensor_tensor(nkb[:], hb[:], -1.0, kk[:],
                                           op0=ALU.mult, op1=ALU.mult)
            nxt = cmpool.tile([D, B], f32r, tag=f"cact{li}")
            nc.scalar.activation(nxt[:], pc[:], AF.Relu,
                                 bias=nkb[:, 0:1], scale=kk[:, 0:1])
            cc = nxt
        pc = pcpool.tile([D, B], f32)
        nc.tensor.matmul(pc[:], wts["w_c3"][:], cc[:],
                         start=True, stop=True)
        c_out = cpool.tile([D, B], f32, tag="c_out")
        nc.scalar.activation(c_out[:], pc[:], AF.Identity,
                             bias=c_b3, scale=1.0)

        # ---- s-MLP pipeline over groups of GRP_S sequence positions ----
        NCHUNK = GCOLS // 512        # 512-col psum chunks per group (4)
        for g in range(NGRP):
            g0 = g * GCOLS
            xg = xpool.tile([D, GCOLS], f32r)
            nc.sync.dma_start(xg[:], xT[:, g0:g0 + GCOLS])

            # ---------- layer 1 ----------
            ps_l1 = []
            st1 = stpool.tile([D, GRP_S, 6], f32, tag="st1")
            for cchunk in range(NCHUNK):
                pt = pspool.tile([D, 512], f32)
                nc.tensor.matmul(pt[:], wts["w_s1"][:],
                                 xg[:, cchunk * 512:(cchunk + 1) * 512],
                                 start=True, stop=True)
                for half in range(2):
                    si = cchunk * 2 + half
                    nc.vector.bn_stats(st1[:, si, :],
                                       pt[:, half * B:(half + 1) * B])
                ps_l1.append(pt)
            hb1, nhb1, kk1 = combine_stats(st1, GRP_S, s_g1, s_beg1, "l1")

            t1 = t1pool.tile([D, GCOLS], f32r)
            for si in range(GRP_S):
                pt = ps_l1[si // 2]
                src = pt[:, (si % 2) * B:(si % 2 + 1) * B]
                dst = t1[:, si * B:(si + 1) * B]
                if si % 4 in NORM_DVE_MOD:
                    nc.vector.tensor_scalar(dst, src, hb1[:, si:si + 1],
                                            hb1[:, si:si + 1],
                                            op0=ALU.max, op1=ALU.subtract)
                else:
                    nc.scalar.activation(dst, src, AF.Relu,
                                         bias=nhb1[:, si:si + 1], scale=1.0)

            # ---------- layer 2 (kk1 folded into weights) ----------
            ps_l2 = []
            st2 = stpool.tile([D, GRP_S, 6], f32, tag="st2")
            for spair in range(GRP_S // 2):
                pt = pspool.tile([D, 512], f32)
                for half in range(2):
                    si = spair * 2 + half
                    wk = wkpool.tile([D, D], f32r, tag="wk2")
                    nc.gpsimd.tensor_scalar(wk[:], wts["w_s2"][:],
                                            kk1[:, si:si + 1], None,
                                            op0=ALU.mult)
                    nc.tensor.matmul(pt[:, half * B:(half + 1) * B], wk[:],
                                     t1[:, si * B:(si + 1) * B],
                                     start=True, stop=True)
                    nc.vector.bn_stats(st2[:, si, :],
                                       pt[:, half * B:(half + 1) * B])
                ps_l2.append(pt)
            hb2, nhb2, kk2 = combine_stats(st2, GRP_S, s_g2, s_beg2, "l2")

            t2 = t2pool.tile([D, GCOLS], f32r)
            for si in range(GRP_S):
                pt = ps_l2[si // 2]
                src = pt[:, (si % 2) * B:(si % 2 + 1) * B]
                dst = t2[:, si * B:(si + 1) * B]
                if si % 4 in NORM_DVE_MOD:
                    nc.vector.tensor_scalar(dst, src, hb2[:, si:si + 1],
                                            hb2[:, si:si + 1],
                                            op0=ALU.max, op1=ALU.subtract)
                else:
                    nc.scalar.activation(dst, src, AF.Relu,
                                         bias=nhb2[:, si:si + 1], scale=1.0)

            # ---------- layer 3 + context gate ----------
            so = sopool.tile([D, GCOLS], f32)
            for spair in range(GRP_S // 2):
                pt = pspool.tile([D, 512], f32)
                for half in range(2):
                    si = spair * 2 + half
                    wk = wkpool.tile([D, D], f32r, tag="wk3")
                    nc.gpsimd.tensor_scalar(wk[:], wts["w_s3"][:],
                                            kk2[:, si:si + 1], None,
                                            op0=ALU.mult)
                    nc.tensor.matmul(pt[:, half * B:(half + 1) * B], wk[:],
                                     t2[:, si * B:(si + 1) * B],
                                     start=True, stop=True)
                dstv = so[:, spair * 512:(spair + 1) * 512]
                if spair % 4 in GATE_DVE_MOD:
                    # (h3 + b3) * c_out, psum-direct on DVE
                    nc.vector.scalar_tensor_tensor(
                        dstv.rearrange("p (s b) -> p s b", b=B),
                        pt[:].rearrange("p (s b) -> p s b", b=B),
                        s_b3,
                        c_out[:].unsqueeze(1).broadcast_to([D, 2, B]),
                        op0=ALU.add, op1=ALU.mult)
                else:
                    # drain+bias on ACT, gate multiply on GPSIMD
                    u = upool.tile([D, 512], f32, tag="u")
                    nc.scalar.activation(u[:], pt[:], AF.Identity,
                                         bias=s_b3, scale=1.0)
                    nc.gpsimd.tensor_tensor(
                        dstv.rearrange("p (s b) -> p s b", b=B),
                        u[:].rearrange("p (s b) -> p s b", b=B),
                        c_out[:].unsqueeze(1).broadcast_to([D, 2, B]),
                        op=ALU.mult)

            nc.sync.dma_start(outT[:, g0:g0 + GCOLS], so[:])

    nc.compile()
    return nc


def _get_nc():
    if "nc" not in _CACHE:
        _CACHE["nc"] = _build_nc()
    return _CACHE["nc"]


def _prep_shard(s, k):
    """s [B, S, D] -> shard [D, S_LOC, B] for core k (blocked transpose)."""
    shard = np.empty((D, S_LOC, B), np.float32)
    base = k * S_LOC
    for i in range(0, S_LOC, 8):
        blk = s[:, base + i:base + i + 8, :]         # [B, 8, D]
        shard[:, i:i + 8, :] = blk.transpose(2, 1, 0)
    return shard


def kernel(**inputs):
    global LAST_RESULTS
    s = np.ascontiguousarray(np.asarray(inputs["s"], dtype=np.float32))
    c = np.asarray(inputs["c"], dtype=np.float32)

    def col(name):
        return np.asarray(inputs[name], dtype=np.float32).reshape(D, 1)

    g1, be1 = col("s_g1"), col("s_be1")
    g2, be2 = col("s_g2"), col("s_be2")
    cg1, cbe1 = col("c_g1"), col("c_be1")
    cg2, cbe2 = col("c_g2"), col("c_be2")
    vecs = np.concatenate([
        g1, be1 / g1, g2, be2 / g2, col("s_b3"),
        cg1, cbe1 / cg1, cg2, cbe2 / cg2, col("c_b3"),
        np.full((D, 1), EPS, np.float32),
    ], axis=1).astype(np.float32)

    consts = dict(
        cT=np.ascontiguousarray(c.T),
        w_s1=np.ascontiguousarray(np.asarray(inputs["s_w1"], np.float32).T),
        w_s2=np.ascontiguousarray(np.asarray(inputs["s_w2"], np.float32).T),
        w_s3=np.ascontiguousarray(np.asarray(inputs["s_w3"], np.float32).T),
        w_c1=np.ascontiguousarray(np.asarray(inputs["c_w1"], np.float32).T),
        w_c2=np.ascontiguousarray(np.asarray(inputs["c_w2"], np.float32).T),
        w_c3=np.ascontiguousarray(np.asarray(inputs["c_w3"], np.float32).T),
        vecs=vecs,
    )

    in_maps = []
    for k in range(NCORES):
        m = dict(consts)
        m["xT"] = _prep_shard(s, k).reshape(D, N_LOC)
        in_maps.append(m)

    nc = _get_nc()
    res = run_bass_kernel_spmd(nc, in_maps, core_ids=list(range(NCORES)))
    LAST_RESULTS = res

    s_out = np.empty((B, S, D), np.float32)
    agg_parts = []
    for k in range(NCORES):
        o = res.results[k]["outT"].reshape(D, S_LOC, B)
        base = k * S_LOC
        for i in range(0, S_LOC, 8):
            s_out[:, base + i:base + i + 8, :] = \
                o[:, i:i + 8, :].transpose(2, 1, 0)
        agg_parts.append(o.max(axis=1))            # [D, B]
    agg = np.maximum.reduce(agg_parts).T           # [B, D]
    return s_out, np.ascontiguousarray(agg)


# revision 5
# speedup vs baseline: 2.0474x; 2.0474x over previous
"""Trainium2 Bass kernel for nn_CGBlock (gnn_message_passing).

Computation (B=256, S=512, D=128):
  c_out = c_mlp(c)                  # [B,D] MLP with BatchNorm over batch
  s_h   = s_mlp(s)                  # [B,S,D] MLP, BN stats over batch axis 0
                                    #   (independent per (seq, feature))
  s_out = s_h * c_out[:, None, :]
  agg   = max(s_out, axis=1)
  returns (s_out, agg)

Sharding: the s-MLP BatchNorm is independent per sequence position, so we
shard the SEQUENCE axis across the 8 cores (64 positions each). Every core
holds the full batch for its slice, so BN needs no cross-device collectives.
The tiny c-MLP is computed redundantly on every core.

On-device layout is feature-major ([D, seq, batch]) so the PE array can
contract over D directly; the host transposes in/out (layout prep only).

BN folding: biases b1/b2 cancel inside BN. BN+ReLU is applied as
  t = relu(h - hb),  hb = mean - be*std/g
with the remaining per-(d,s) scale kk = g/std folded into the next layer's
weights (w_next * kk per sequence position), since relu commutes with the
positive scale kk (requires g > 0, true for these inputs).
"""
import sys
import os

for _p in ('/opt/trn_rl_repo', '/root/.axon_site/_ro/trn_rl_repo'):
    if os.path.isdir(_p) and _p not in sys.path:
        sys.path.insert(0, _p)

import numpy as np
from contextlib import ExitStack

import concourse.bass as bass
import concourse.bacc as bacc
import concourse.tile as tile
import concourse.mybir as mybir
from concourse.bass_utils import run_bass_kernel_spmd

f32 = mybir.dt.float32
f32r = mybir.dt.float32r
AF = mybir.ActivationFunctionType
ALU = mybir.AluOpType

B, S, D = 256, 512, 128
NCORES = 8
S_LOC = S // NCORES          # 64 sequence positions per core
N_LOC = S_LOC * B            # 16384 columns per core
GRP_S = 8                    # sequence positions per pipeline group
NGRP = S_LOC // GRP_S        # 8 groups
GCOLS = GRP_S * B            # 2048 columns per group
EPS = 1e-5

# Tuning knobs.
NORM_DVE_MOD = ()            # s-indices (mod 4) normalized on DVE instead of ACT
GATE_DVE_MOD = (0, 2)        # 512-col chunks (mod 4) gated on DVE direct path

LAST_RESULTS = None
_CACHE = {}


def _build_nc():
    nc = bacc.Bacc("TRN2", target_bir_lowering=False, debug=False)

    xT = nc.dram_tensor("xT", [D, N_LOC], f32r, kind="ExternalInput").ap()
    cT = nc.dram_tensor("cT", [D, B], f32r, kind="ExternalInput").ap()
    # weights pre-transposed on host: [in_feature, out_feature]
    wnames = ["w_s1", "w_s2", "w_s3", "w_c1", "w_c2", "w_c3"]
    wap = {n: nc.dram_tensor(n, [D, D], f32r, kind="ExternalInput").ap()
           for n in wnames}
    # packed per-partition vectors:
    # [s_g1, s_beg1, s_g2, s_beg2, s_b3, c_g1, c_beg1, c_g2, c_beg2, c_b3, EPS]
    vecs = nc.dram_tensor("vecs", [D, 11], f32, kind="ExternalInput").ap()

    outT = nc.dram_tensor("outT", [D, N_LOC], f32, kind="ExternalOutput").ap()

    with tile.TileContext(nc) as tc, ExitStack() as ctx:
        cpool = ctx.enter_context(tc.tile_pool(name="consts", bufs=1))
        xpool = ctx.enter_context(tc.tile_pool(name="xin", bufs=3))
        t1pool = ctx.enter_context(tc.tile_pool(name="t1", bufs=2))
        t2pool = ctx.enter_context(tc.tile_pool(name="t2", bufs=2))
        sopool = ctx.enter_context(tc.tile_pool(name="sout", bufs=3))
        upool = ctx.enter_context(tc.tile_pool(name="udrain", bufs=4))
        stpool = ctx.enter_context(tc.tile_pool(name="stats", bufs=3))
        smpool = ctx.enter_context(tc.tile_pool(name="smalls", bufs=3))
        cmpool = ctx.enter_context(tc.tile_pool(name="cmlp", bufs=2))
        pspool = ctx.enter_context(tc.tile_pool(name="ps", bufs=6, space="PSUM"))
        pcpool = ctx.enter_context(tc.tile_pool(name="psc", bufs=2, space="PSUM"))

        # ---- load constants ----
        wts = {}
        for nm in wnames:
            t = cpool.tile([D, D], f32r, tag=nm)
            nc.sync.dma_start(t[:], wap[nm])
            wts[nm] = t
        ct = cpool.tile([D, B], f32r, tag="ct")
        nc.sync.dma_start(ct[:], cT)
        vt = cpool.tile([D, 11], f32, tag="vecs")
        nc.sync.dma_start(vt[:], vecs)
        s_g1 = vt[:, 0:1]; s_beg1 = vt[:, 1:2]
        s_g2 = vt[:, 2:3]; s_beg2 = vt[:, 3:4]
        s_b3 = vt[:, 4:5]
        c_g1 = vt[:, 5:6]; c_beg1 = vt[:, 6:7]
        c_g2 = vt[:, 7:8]; c_beg2 = vt[:, 8:9]
        c_b3 = vt[:, 9:10]
        eps_ap = vt[:, 10:11]

        # ---- helper: combine bn_stats (even/odd) into hb / nhb / kk ----
        def combine_stats(st, n, g_ap, beg_ap, tag):
            me = st[:, :, 1]; mo = st[:, :, 4]
            cve = st[:, :, 2]; cvo = st[:, :, 5]
            Ssum = smpool.tile([D, n], f32, tag=f"S_{tag}")
            nc.gpsimd.tensor_tensor(Ssum[:], me, mo, op=ALU.add)
            Dd = smpool.tile([D, n], f32, tag=f"D_{tag}")
            nc.gpsimd.tensor_tensor(Dd[:], me, mo, op=ALU.subtract)
            Q = smpool.tile([D, n], f32, tag=f"Q_{tag}")
            nc.gpsimd.tensor_tensor(Q[:], cve, cvo, op=ALU.add)
            D2 = smpool.tile([D, n], f32, tag=f"D2_{tag}")
            nc.gpsimd.tensor_tensor(D2[:], Dd[:], Dd[:], op=ALU.mult)
            Qs = smpool.tile([D, n], f32, tag=f"Qs_{tag}")
            nc.vector.tensor_scalar(Qs[:], Q[:], 1.0 / B, None, op0=ALU.mult)
            veps = smpool.tile([D, n], f32, tag=f"veps_{tag}")
            nc.vector.scalar_tensor_tensor(veps[:], D2[:], 0.25, Qs[:],
                                           op0=ALU.mult, op1=ALU.add)
            std = smpool.tile([D, n], f32, tag=f"std_{tag}")
            nc.scalar.activation(std[:], veps[:], AF.Sqrt, bias=eps_ap, scale=1.0)
            rstd = smpool.tile([D, n], f32, tag=f"rstd_{tag}")
            nc.vector.reciprocal(rstd[:], std[:])
            kk = smpool.tile([D, n], f32, tag=f"kk_{tag}")
            nc.vector.tensor_scalar(kk[:], rstd[:], g_ap, None, op0=ALU.mult)
            bestd = smpool.tile([D, n], f32, tag=f"bestd_{tag}")
            nc.vector.tensor_scalar(bestd[:], std[:], beg_ap, None, op0=ALU.mult)
            hb = smpool.tile([D, n], f32, tag=f"hb_{tag}")
            nc.vector.scalar_tensor_tensor(hb[:], Ssum[:], 0.5, bestd[:],
                                           op0=ALU.mult, op1=ALU.subtract)
            nkb = smpool.tile([D, n], f32, tag=f"nkb_{tag}")
            nc.vector.scalar_tensor_tensor(nkb[:], hb[:], -1.0, kk[:],
                                           op0=ALU.mult, op1=ALU.mult)
            return hb, nkb, kk

        # ---- c-MLP (tiny, replicated on every core) ----
        cc = ct
        for li, (wn, g_ap, beg_ap) in enumerate(
                [("w_c1", c_g1, c_beg1), ("w_c2", c_g2, c_beg2)]):
            pc = pcpool.tile([D, B], f32)
            nc.tensor.matmul(pc[:], wts[wn][:], cc[:],
                             start=True, stop=True)
            st = stpool.tile([D, 1, 6], f32, tag=f"stc{li}")
            nc.vector.bn_stats(st[:, 0, :], pc[:])
            hb, nkb, kk = combine_stats(st, 1, g_ap, beg_ap, f"c{li}")
            nxt = cmpool.tile([D, B], f32r, tag=f"cact{li}")
            nc.scalar.activation(nxt[:], pc[:], AF.Relu,
                                 bias=nkb[:, 0:1], scale=kk[:, 0:1])
            cc = nxt
        pc = pcpool.tile([D, B], f32)
        nc.tensor.matmul(pc[:], wts["w_c3"][:], cc[:],
                         start=True, stop=True)
        c_out = cpool.tile([D, B], f32, tag="c_out")
        nc.scalar.activation(c_out[:], pc[:], AF.Identity,
                             bias=c_b3, scale=1.0)

        # ---- s-MLP pipeline over groups of GRP_S sequence positions ----
        NCHUNK = GCOLS // 512        # 512-col psum chunks per group (4)
        for g in range(NGRP):
            g0 = g * GCOLS
            xg = xpool.tile([D, GCOLS], f32r)
            nc.sync.dma_start(xg[:], xT[:, g0:g0 + GCOLS])

            # ---------- layer 1 ----------
            ps_l1 = []
            st1 = stpool.tile([D, GRP_S, 6], f32, tag="st1")
            for cchunk in range(NCHUNK):
                pt = pspool.tile([D, 512], f32)
                nc.tensor.matmul(pt[:], wts["w_s1"][:],
                                 xg[:, cchunk * 512:(cchunk + 1) * 512],
                                 start=True, stop=True)
                for half in range(2):
                    si = cchunk * 2 + half
                    nc.vector.bn_stats(st1[:, si, :],
                                       pt[:, half * B:(half + 1) * B])
                ps_l1.append(pt)
            hb1, nkb1, kk1 = combine_stats(st1, GRP_S, s_g1, s_beg1, "l1")

            t1 = t1pool.tile([D, GCOLS], f32r)
            for si in range(GRP_S):
                pt = ps_l1[si // 2]
                src = pt[:, (si % 2) * B:(si % 2 + 1) * B]
                dst = t1[:, si * B:(si + 1) * B]
                nc.scalar.activation(dst, src, AF.Relu,
                                     bias=nkb1[:, si:si + 1],
                                     scale=kk1[:, si:si + 1])

            # ---------- layer 2 (kk1 folded into weights) ----------
            ps_l2 = []
            st2 = stpool.tile([D, GRP_S, 6], f32, tag="st2")
            for cchunk in range(NCHUNK):
                pt = pspool.tile([D, 512], f32)
                nc.tensor.matmul(pt[:], wts["w_s2"][:],
                                 t1[:, cchunk * 512:(cchunk + 1) * 512],
                                 start=True, stop=True)
                for half in range(2):
                    si = cchunk * 2 + half
                    nc.vector.bn_stats(st2[:, si, :],
                                       pt[:, half * B:(half + 1) * B])
                ps_l2.append(pt)
            hb2, nkb2, kk2 = combine_stats(st2, GRP_S, s_g2, s_beg2, "l2")

            t2 = t2pool.tile([D, GCOLS], f32r)
            for si in range(GRP_S):
                pt = ps_l2[si // 2]
                src = pt[:, (si % 2) * B:(si % 2 + 1) * B]
                dst = t2[:, si * B:(si + 1) * B]
                nc.scalar.activation(dst, src, AF.Relu,
                                     bias=nkb2[:, si:si + 1],
                                     scale=kk2[:, si:si + 1])

            # ---------- layer 3 + context gate ----------
            so = sopool.tile([D, GCOLS], f32)
            for spair in range(GRP_S // 2):
                pt = pspool.tile([D, 512], f32)
                nc.tensor.matmul(pt[:], wts["w_s3"][:],
                                 t2[:, spair * 512:(spair + 1) * 512],
                                 start=True, stop=True)
                dstv = so[:, spair * 512:(spair + 1) * 512]
                if spair % 4 in GATE_DVE_MOD:
                    # (h3 + b3) * c_out, psum-direct on DVE
                    nc.vector.scalar_tensor_tensor(
                        dstv.rearrange("p (s b) -> p s b", b=B),
                        pt[:].rearrange("p (s b) -> p s b", b=B),
                        s_b3,
                        c_out[:].unsqueeze(1).broadcast_to([D, 2, B]),
                        op0=ALU.add, op1=ALU.mult)
                else:
                    # drain+bias on ACT, gate multiply on GPSIMD
                    u = upool.tile([D, 512], f32, tag="u")
                    nc.scalar.activation(u[:], pt[:], AF.Identity,
                                         bias=s_b3, scale=1.0)
                    nc.gpsimd.tensor_tensor(
                        dstv.rearrange("p (s b) -> p s b", b=B),
                        u[:].rearrange("p (s b) -> p s b", b=B),
                        c_out[:].unsqueeze(1).broadcast_to([D, 2, B]),
                        op=ALU.mult)

            nc.sync.dma_start(outT[:, g0:g0 + GCOLS], so[:])

    nc.compile()
    return nc


def _get_nc():
    if "nc" not in _CACHE:
        _CACHE["nc"] = _build_nc()
    return _CACHE["nc"]


def _prep_shard(s, k):
    """s [B, S, D] -> shard [D, S_LOC, B] for core k (blocked transpose)."""
    shard = np.empty((D, S_LOC, B), np.float32)
    base = k * S_LOC
    for i in range(0, S_LOC, 8):
        blk = s[:, base + i:base + i + 8, :]         # [B, 8, D]
        shard[:, i:i + 8, :] = blk.transpose(2, 1, 0)
    return shard


def kernel(**inputs):
    global LAST_RESULTS
    s = np.ascontiguousarray(np.asarray(inputs["s"], dtype=np.float32))
    c = np.asarray(inputs["c"], dtype=np.float32)

    def col(name):
        return np.asarray(inputs[name], dtype=np.float32).reshape(D, 1)

    g1, be1 = col("s_g1"), col("s_be1")
    g2, be2 = col("s_g2"), col("s_be2")
    cg1, cbe1 = col("c_g1"), col("c_be1")
    cg2, cbe2 = col("c_g2"), col("c_be2")
    vecs = np.concatenate([
        g1, be1 / g1, g2, be2 / g2, col("s_b3"),
        cg1, cbe1 / cg1, cg2, cbe2 / cg2, col("c_b3"),
        np.full((D, 1), EPS, np.float32),
    ], axis=1).astype(np.float32)

    consts = dict(
        cT=np.ascontiguousarray(c.T),
        w_s1=np.ascontiguousarray(np.asarray(inputs["s_w1"], np.float32).T),
        w_s2=np.ascontiguousarray(np.asarray(inputs["s_w2"], np.float32).T),
        w_s3=np.ascontiguousarray(np.asarray(inputs["s_w3"], np.float32).T),
        w_c1=np.ascontiguousarray(np.asarray(inputs["c_w1"], np.float32).T),
        w_c2=np.ascontiguousarray(np.asarray(inputs["c_w2"], np.float32).T),
        w_c3=np.ascontiguousarray(np.asarray(inputs["c_w3"], np.float32).T),
        vecs=vecs,
    )

    in_maps = []
    for k in range(NCORES):
        m = dict(consts)
        m["xT"] = _prep_shard(s, k).reshape(D, N_LOC)
        in_maps.append(m)

    nc = _get_nc()
    res = run_bass_kernel_spmd(nc, in_maps, core_ids=list(range(NCORES)))
    LAST_RESULTS = res

    s_out = np.empty((B, S, D), np.float32)
    agg_parts = []
    for k in range(NCORES):
        o = res.results[k]["outT"].reshape(D, S_LOC, B)
        base = k * S_LOC
        for i in range(0, S_LOC, 8):
            s_out[:, base + i:base + i + 8, :] = \
                o[:, i:i + 8, :].transpose(2, 1, 0)
        agg_parts.append(o.max(axis=1))            # [D, B]
    agg = np.maximum.reduce(agg_parts).T           # [B, D]
    return s_out, np.ascontiguousarray(agg)
